# revision 16
# baseline (speedup 1.0000x reference)
"""Trainium2 Bass kernel for nn_Decoder (additive-attention LSTM decoder).

v2: two phase-shifted batch streams of 64 rows per core. While stream A runs
its z3 = tanh(z1+z2) bulk phase (ACT-bound), stream B runs its serial phase
(softmax -> LSTM update -> next z1) on the other engines, so the scalar
engine never idles. Elementwise work is spread over ACT (native tanh),
DVE (clamped odd-polynomial tanh at 2x/4x fp16 rates) and the otherwise-idle
Pool engine (broadcast adds). Scores use t-pair-packed stationaries
([128e, 2t x 64b]) so the PE matmul count stays at baseline level.
"""

import os
import numpy as np

B, T, E, D = 1024, 64, 512, 512
NCORES = 8
BL = B // NCORES          # 128 batch rows per core
SB = 64                   # rows per stream (2 streams per core)
EC = E // 128             # 4 e-chunks
KD = (2 * D) // 128       # 8 contraction chunks for z1
GB = (4 * D) // 128       # 16 gate blocks
NCH = 8                   # z3 chunks per stream-step: (half, c)
N_STEPS = int(os.environ.get("KERNEL_N_STEPS", str(T)))

# tanh polynomial (filled from calibration): tanh(x) ~ ((C7*u+C5)*u+C3)*u*x
# + C1*x with u = clamp(x)^2  -- see calib.py
CLAMP = 4.0
PCOEF = (9.997635e-01, -3.243438e-01, 7.985914e-02, -1.240508e-02,
         7.850441e-04)  # c1, c3, c5, c7, c9 (degree 9; trim as configured)
POLY_DEG = 7

# which chunks (i = half*4 + c) use DVE-poly tanh instead of ACT tanh
DVE_POLY_CHUNKS = ()
# which chunk adds go on the gpsimd DSP (rest on DVE) -- gpsimd runs
# tensor_tensor at 0.42 efficiency (~4.2us/chunk) but is otherwise idle
POOL_ADD_CHUNKS = (1, 5)

_PROG_CACHE = {}


def _build_program(n_steps, bff):
    from contextlib import ExitStack

    import concourse.bass as bass
    import concourse.tile as tile
    from concourse import bacc, mybir

    f16 = mybir.dt.float16
    f32 = mybir.dt.float32
    AF = mybir.ActivationFunctionType
    OP = mybir.AluOpType

    nc = bacc.Bacc("TRN2", target_bir_lowering=False, debug=False)

    xtA_d = nc.dram_tensor("xtA", (128, EC * T * SB), f16, kind="ExternalInput")
    xtB_d = nc.dram_tensor("xtB", (128, EC * T * SB), f16, kind="ExternalInput")
    ytwt_d = nc.dram_tensor("ytwt", (1, 2 * T * SB), f32,
                            kind="ExternalInput")
    wa1_d = nc.dram_tensor("wa1t", (128, KD * 512), f16, kind="ExternalInput")
    wa2_d = nc.dram_tensor("wa2t", (128, EC * 512), f16, kind="ExternalInput")
    wa3_d = nc.dram_tensor("wa3", (128, EC), f16, kind="ExternalInput")
    whh_d = nc.dram_tensor("whht", (128, 4 * 2048), f16, kind="ExternalInput")
    wihb_d = nc.dram_tensor("wihb", (2, 2048), f16, kind="ExternalInput")
    bias1_d = nc.dram_tensor("bias1", (128, EC), f32, kind="ExternalInput")
    wffh_d = nc.dram_tensor("wffh", (128, EC), f16, kind="ExternalInput")
    xwp_d = nc.dram_tensor("xwp", (128, 64), f32, kind="ExternalInput")
    xw2p_d = nc.dram_tensor("xw2p", (128, 64), f32, kind="ExternalInput")
    ident_d = nc.dram_tensor("ident", (128, 128), f32, kind="ExternalInput")
    fold_d = nc.dram_tensor("fold64", (128, SB), f32, kind="ExternalInput")
    outA_d = nc.dram_tensor("outA", (SB, 1), f32, kind="ExternalOutput")
    outB_d = nc.dram_tensor("outB", (SB, 1), f32, kind="ExternalOutput")

    with tile.TileContext(nc) as tc, ExitStack() as ctx:
        const = ctx.enter_context(tc.tile_pool(name="const", bufs=1))
        z2pool = ctx.enter_context(tc.tile_pool(name="z2pool", bufs=1))
        state = ctx.enter_context(tc.tile_pool(name="state", bufs=1))

        # ---- constants into SBUF ----
        wa1t = const.tile([128, KD * 512], f16, name="wa1t", tag="wa1t")
        nc.sync.dma_start(wa1t[:], wa1_d.ap())
        whht = const.tile([128, 4 * 2048], f16, name="whht", tag="whht")
        nc.sync.dma_start(whht[:], whh_d.ap())
        wa3s = const.tile([128, EC], f16, name="wa3s", tag="wa3s")
        nc.sync.dma_start(wa3s[:], wa3_d.ap())
        wihb = const.tile([2, 2048], f16, name="wihb", tag="wihb")
        nc.sync.dma_start(wihb[:], wihb_d.ap())
        bias1 = const.tile([128, EC], f32, name="bias1", tag="bias1")
        nc.sync.dma_start(bias1[:], bias1_d.ap())
        wffh = const.tile([128, EC], f16, name="wffh", tag="wffh")
        nc.sync.dma_start(wffh[:], wffh_d.ap())
        ident = const.tile([128, 128], f32, name="ident", tag="ident")
        nc.sync.dma_start(ident[:], ident_d.ap())
        fold64 = const.tile([128, SB], f32, name="fold64", tag="fold64")
        nc.sync.dma_start(fold64[:], fold_d.ap())
        ytwt = const.tile([1, 2 * T * SB], f32, name="ytwt", tag="ytwt")
        nc.sync.dma_start(ytwt[:], ytwt_d.ap())
        xwp = const.tile([128, 64], f32, name="xwp", tag="xwp")
        nc.sync.dma_start(xwp[:], xwp_d.ap())
        xw2p = const.tile([128, 64], f32, name="xw2p", tag="xw2p")
        nc.sync.dma_start(xw2p[:], xw2p_d.ap())

        # z2 per stream: [p=e-feat, c*4096 + t*64 + b]
        z2t = {}
        z2t[0] = z2pool.tile([128, EC * T * SB], f16, name="z2A", tag="z2A")
        z2t[1] = z2pool.tile([128, EC * T * SB], f16, name="z2B", tag="z2B")

        # ---- precompute phase: z2 = x @ W_a2.T (+ b folded into z1p later)
        with tc.tile_pool(name="xtp", bufs=1) as xtp, \
             tc.tile_pool(name="pcps", bufs=4, space="PSUM") as pcps:
            xts = {}
            xts[0] = xtp.tile([128, EC * T * SB], f16, name="xtsA", tag="xtsA")
            nc.sync.dma_start(xts[0][:], xtA_d.ap())
            xts[1] = xtp.tile([128, EC * T * SB], f16, name="xtsB", tag="xtsB")
            nc.sync.dma_start(xts[1][:], xtB_d.ap())
            wa2t = xtp.tile([128, EC * 512], f16, name="wa2t", tag="wa2t")
            nc.sync.dma_start(wa2t[:], wa2_d.ap())

            evac = 0
            for S in (0, 1):
                for cf in range(EC):
                    for n in range(8):
                        zp = pcps.tile([128, 512], f32, name="zp", tag="zp")
                        for k in range(EC):
                            nc.tensor.matmul(
                                zp[:],
                                wa2t[:, k * 512 + cf * 128:
                                     k * 512 + (cf + 1) * 128],
                                xts[S][:, k * 4096 + n * 512:
                                       k * 4096 + (n + 1) * 512],
                                start=(k == 0), stop=(k == EC - 1))
                        dst = z2t[S][:, cf * 4096 + n * 512:
                                     cf * 4096 + (n + 1) * 512]
                        if evac % 2 == 0:
                            nc.vector.tensor_copy(dst, zp[:])
                        else:
                            nc.scalar.copy(dst, zp[:])
                        evac += 1

        # ---- loop-phase pools ----
        z3pool = ctx.enter_context(tc.tile_pool(name="z3pool", bufs=4))
        polyp = ctx.enter_context(tc.tile_pool(name="polyp", bufs=2))
        work = ctx.enter_context(tc.tile_pool(name="work", bufs=3))
        psA = ctx.enter_context(tc.tile_pool(name="psA", bufs=1, space="PSUM"))
        psB = ctx.enter_context(tc.tile_pool(name="psB", bufs=1, space="PSUM"))
        gpsA = ctx.enter_context(
            tc.tile_pool(name="gpsA", bufs=1, space="PSUM"))
        gpsB = ctx.enter_context(
            tc.tile_pool(name="gpsB", bufs=1, space="PSUM"))

        # ---- per-stream state ----
        streams = []
        for S, (ps, gp, out_d) in enumerate(
                ((psA, gpsA, outA_d), (psB, gpsB, outB_d))):
            pst = ps.tile([128, 512], f32, name=f"pst{S}", tag="pst")
            st = {
                "id": S, "bh": S * SB, "off": S * 32,
                "ps": ps, "gp": gp, "out_d": out_d, "z2": z2t[S],
                "z1ps": pst[:, 0:256], "scps": pst[:, 256:288],
                "dnps": pst[0:1, 288:352], "ynps": pst[0:1, 352:416],
                "a2ps": pst[0:1, 64:128], "obps": pst[0:1, 0:64],
                "hT": state.tile([128, 4 * SB], f16, name=f"hT{S}",
                                 tag=f"hT{S}"),
                "cD": state.tile([128, 4 * SB], f32, name=f"cD{S}",
                                 tag=f"cD{S}"),
                "cT16": state.tile([128, 4 * SB], f16, name=f"cT16{S}",
                                   tag=f"cT16{S}"),
                "ytones": state.tile([2, SB], f16, name=f"ytones{S}",
                                     tag=f"ytones{S}"),
                "z1p": state.tile([128, 4 * SB], f16, name=f"z1p{S}",
                                  tag=f"z1p{S}"),
                "e_sc": state.tile([128, 32], f32, name=f"esc{S}",
                                   tag=f"esc{S}"),
                "rden": state.tile([1, SB], f32, name=f"rden{S}",
                                   tag=f"rden{S}"),
                "gps_t": None, "tifo": None, "tg": None,
            }
            nc.vector.memset(st["hT"][:], 0.0)
            nc.vector.memset(st["cD"][:], 0.0)
            nc.vector.memset(st["cT16"][:], 0.0)
            nc.vector.memset(st["ytones"][:], 1.0)
            streams.append(st)

        id64 = ident[0:SB, 0:SB]

        # ---------- emission helpers ----------

        def ser_softmax(st, s):
            """softmax of step s (scores already in st.scps), y_tilde as a
            [1, 64] row, close gates psum with W_ih matmuls."""
            e_sc, scps = st["e_sc"], st["scps"]
            den_p = work.tile([128, 1], f32, name="den_p", tag="den_p")
            nc.scalar.activation(e_sc[:], scps, AF.Exp,
                                 accum_out=den_p[:])
            tmp = work.tile([128, 32], f32, name="tmp", tag="tmp")
            ynum_p = work.tile([128, 1], f32, name="ynum_p", tag="ynum_p")
            nc.vector.scalar_tensor_tensor(
                tmp[:], e_sc[:], 1.0, xwp[:, st["off"]:st["off"] + 32],
                OP.bypass, OP.mult, accum_out=ynum_p[:])
            nc.tensor.matmul(st["dnps"], den_p[:], fold64[:],
                             start=True, stop=True)
            nc.tensor.matmul(st["ynps"], ynum_p[:], fold64[:],
                             start=True, stop=True)
            nc.vector.reciprocal(st["rden"][:], st["dnps"])
            ytmp = work.tile([1, SB], f32, name="ytmp", tag="ytmp")
            nc.vector.tensor_tensor(ytmp[:], st["ynps"], st["rden"][:],
                                    op=OP.mult)
            nc.vector.tensor_tensor(
                st["ytones"][0:1, :], ytmp[:],
                ytwt[0:1, st["id"] * T * SB + s * SB:
                     st["id"] * T * SB + (s + 1) * SB], op=OP.add)
            gps = st["gps_t"]
            for m in range(GB):
                nc.tensor.matmul(
                    gps[:, m * SB:(m + 1) * SB],
                    wihb[:, m * 128:(m + 1) * 128],
                    st["ytones"][:], start=False, stop=(m % 8 == 7))

        def ser_gate_tanh(st):
            gps = st["gps_t"]
            tifo = work.tile([128, 12 * SB], f32, name="tifo", tag="tifo")
            nc.scalar.activation(tifo[:], gps[:, 0:12 * SB], AF.Tanh,
                                 scale=0.5)
            tg = work.tile([128, 4 * SB], f32, name="tg", tag="tg")
            nc.scalar.activation(tg[:], gps[:, 12 * SB:16 * SB], AF.Tanh)
            st["tifo"], st["tg"] = tifo, tg

        def bulk_chunk(st, i, first, last):
            c, half = i % EC, i // EC
            base = c * T * SB + half * (T // 2) * SB
            z3t = z3pool.tile([128, (T // 2) * SB], f16, name="z3t",
                              tag="z3t")
            add_eng = nc.gpsimd if i in POOL_ADD_CHUNKS else nc.vector
            add_eng.tensor_tensor(
                z3t.rearrange("p (t b) -> p t b", t=T // 2),
                st["z2"][:, base:base + (T // 2) * SB]
                .rearrange("p (t b) -> p t b", t=T // 2),
                st["z1p"][:, c * SB:(c + 1) * SB].unsqueeze(1)
                .broadcast_to((128, T // 2, SB)),
                op=OP.add)
            if i in DVE_POLY_CHUNKS:
                _poly_tanh(z3t)
            else:
                nc.scalar.activation(z3t[:], z3t[:], AF.Tanh)
            scps = st["scps"]
            for tp in range(16):
                col = half * 16 + tp
                nc.tensor.matmul(
                    scps[:, col:col + 1],
                    z3t[:, tp * 128:(tp + 1) * 128],
                    wa3s[:, c:c + 1],
                    start=(first and tp == 0), stop=(last and tp == 15))

        def _poly_tanh(z3t):
            xc = polyp.tile([128, (T // 2) * SB], f16, name="xc", tag="xc")
            nc.vector.tensor_scalar(xc[:], z3t[:], -CLAMP, CLAMP,
                                    OP.max, OP.min)
            u = polyp.tile([128, (T // 2) * SB], f16, name="u", tag="u")
            nc.vector.tensor_tensor(u[:], xc[:], xc[:], op=OP.mult)
            h = polyp.tile([128, (T // 2) * SB], f16, name="h", tag="h")
            c1, c3, c5, c7 = PCOEF[0], PCOEF[1], PCOEF[2], PCOEF[3]
            if POLY_DEG == 7:
                nc.vector.tensor_scalar(h[:], u[:], c7, c5, OP.mult, OP.add)
                nc.vector.tensor_tensor(h[:], h[:], u[:], op=OP.mult)
                nc.vector.tensor_scalar(h[:], h[:], c3, None, OP.add)
            else:
                nc.vector.tensor_scalar(h[:], u[:], c5, c3, OP.mult, OP.add)
            nc.vector.tensor_tensor(h[:], h[:], u[:], op=OP.mult)
            nc.vector.tensor_scalar(h[:], h[:], c1, None, OP.add)
            nc.vector.tensor_tensor(z3t[:], h[:], xc[:], op=OP.mult)

        def ser_z1(st, s):
            z1ps = st["z1ps"]
            nmm = 0
            for k in list(range(4, KD)) + list(range(4)):
                rhs = (st["hT"][:, (k) * SB:(k + 1) * SB] if k < 4 else
                       st["cT16"][:, (k - 4) * SB:(k - 3) * SB])
                for m in range(EC):
                    nc.tensor.matmul(
                        z1ps[:, m * SB:(m + 1) * SB],
                        wa1t[:, k * 512 + m * 128:k * 512 + (m + 1) * 128],
                        rhs, start=(nmm == 0), stop=(nmm == KD * EC - 1))
                    nmm += 1
            gps = st["gp"].tile([128, GB * SB], f32, name="gps", tag="gps")
            st["gps_t"] = gps
            for m in range(GB):
                for k in range(4):
                    nc.tensor.matmul(
                        gps[:, m * SB:(m + 1) * SB],
                        whht[:, k * 2048 + m * 128:k * 2048 + (m + 1) * 128],
                        st["hT"][:, k * SB:(k + 1) * SB],
                        start=(k == 0 and m % 8 == 0), stop=False)
            nc.vector.tensor_tensor(
                st["z1p"].rearrange("p (m b) -> p m b", m=EC),
                z1ps.rearrange("p (m b) -> p m b", m=EC),
                bias1.unsqueeze(2).broadcast_to((128, EC, SB)),
                op=OP.add)

        def ser_state2(st):
            tifo, tg = st["tifo"], st["tg"]
            w = 4 * SB
            t1 = work.tile([128, w], f32, name="t1", tag="t1")
            nc.vector.scalar_tensor_tensor(
                t1[:], tifo[:, w:2 * w], 1.0, st["cD"][:], OP.add, OP.mult)
            t2 = work.tile([128, w], f32, name="t2", tag="t2")
            nc.vector.scalar_tensor_tensor(
                t2[:], tifo[:, 0:w], 1.0, tg[:], OP.add, OP.mult)
            nc.vector.scalar_tensor_tensor(
                st["cD"][:], t1[:], 0.5, t2[:], OP.mult, OP.add)
            tcn = work.tile([128, w], f32, name="tcn", tag="tcn")
            nc.scalar.activation(tcn[:], st["cD"][:], AF.Tanh, scale=0.5)
            nc.vector.scalar_tensor_tensor(
                st["hT"][:], tifo[:, 2 * w:3 * w], 1.0, tcn[:],
                OP.add, OP.mult)
            nc.vector.tensor_copy(st["cT16"][:], st["cD"][:])

        def emit_slot(bulk_st, ser_st, s_bulk, s_ser, last_ser):
            if s_ser > 0:
                ser_softmax(ser_st, s_ser - 1)
            bulk_chunk(bulk_st, 0, True, False)
            if s_ser > 0:
                ser_gate_tanh(ser_st)
            bulk_chunk(bulk_st, 1, False, False)
            if s_ser > 0:
                ser_state2(ser_st)
            bulk_chunk(bulk_st, 2, False, False)
            if not last_ser:
                ser_z1(ser_st, s_ser)
            for i in range(3, NCH):
                bulk_chunk(bulk_st, i, False, i == NCH - 1)

        def final_out(st):
            obps = st["obps"]
            for k in range(EC):
                nc.tensor.matmul(obps, wffh[:, k:k + 1],
                                 st["hT"][:, k * SB:(k + 1) * SB],
                                 start=(k == 0), stop=(k == EC - 1))
            tmpf = work.tile([128, 32], f32, name="tmpf", tag="tmp")
            a2n_p = work.tile([128, 1], f32, name="a2n_p", tag="den_p")
            nc.vector.scalar_tensor_tensor(
                tmpf[:], st["e_sc"][:], 1.0,
                xw2p[:, st["off"]:st["off"] + 32],
                OP.bypass, OP.mult, accum_out=a2n_p[:])
            nc.tensor.matmul(st["a2ps"], a2n_p[:], fold64[:],
                             start=True, stop=True)
            a2 = work.tile([1, SB], f32, name="a2", tag="ytmp")
            nc.vector.tensor_tensor(a2[:], st["a2ps"], st["rden"][:],
                                    op=OP.mult)
            osb = work.tile([1, SB], f32, name="osb", tag="osb")
            nc.vector.scalar_tensor_tensor(
                osb[:], obps, float(bff), a2[:], OP.add, OP.add)
            nc.sync.dma_start(st["out_d"].ap(), osb[:])

        # ---------- main loop ----------
        A, Bs = streams[0], streams[1]
        ser_z1(A, 0)
        for s in range(n_steps):
            emit_slot(A, Bs, s, s, last_ser=False)
            # last odd slot still emits A's softmax/LSTM tail (s_ser>0),
            # only ser_z1 is skipped via last_ser
            emit_slot(Bs, A, s, s + 1, last_ser=(s == n_steps - 1))
        # B's tail
        ser_softmax(Bs, n_steps - 1)
        ser_gate_tanh(Bs)
        ser_state2(Bs)
        final_out(A)
        final_out(Bs)

    nc.compile()
    return nc


def _prep_inputs(inputs):
    """Host-side layout prep. Returns list of per-core input dicts."""
    f16 = np.float16
    x = np.asarray(inputs["input_encoded"], dtype=np.float32)
    yh = np.asarray(inputs["y_history"], dtype=np.float32)
    W_a1 = np.asarray(inputs["W_a1"], dtype=np.float32)
    b_a1 = np.asarray(inputs["b_a1"], dtype=np.float32)
    W_a2 = np.asarray(inputs["W_a2"], dtype=np.float32)
    b_a2 = np.asarray(inputs["b_a2"], dtype=np.float32)
    W_a3 = np.asarray(inputs["W_a3"], dtype=np.float32)
    W_ih = np.asarray(inputs["W_ih"], dtype=np.float32)
    W_hh = np.asarray(inputs["W_hh"], dtype=np.float32)
    b_ih = np.asarray(inputs["b_ih"], dtype=np.float32)
    b_hh = np.asarray(inputs["b_hh"], dtype=np.float32)
    W_fc = np.asarray(inputs["W_fc"], dtype=np.float32)
    W_ff = np.asarray(inputs["W_ff"], dtype=np.float32)

    order = np.r_[0:512, 512:1024, 1536:2048, 1024:1536]  # [i, f, o, g]

    wa1t = ((W_a1.T / 2).reshape(KD, 128, 512).transpose(1, 0, 2)
            .reshape(128, KD * 512).astype(f16))
    wa2t = (W_a2.T.reshape(EC, 128, 512).transpose(1, 0, 2)
            .reshape(128, EC * 512).astype(f16))
    wa3 = W_a3[0].reshape(EC, 128).T.astype(f16).copy()
    whht = ((W_hh[order] / 2).T.reshape(4, 128, 2048).transpose(1, 0, 2)
            .reshape(128, 4 * 2048).astype(f16))
    wihb = np.stack([W_ih[order, 0], (b_ih + b_hh)[order]]).astype(f16)
    bias1 = (b_a1 + b_a2).reshape(EC, 128).T.astype(np.float32).copy()
    wffh = (W_ff[0, :512] / 2).reshape(EC, 128).T.astype(f16).copy()
    ident = np.eye(128, dtype=np.float32)
    fold64 = np.concatenate([np.eye(SB, dtype=np.float32)] * 2, axis=0)

    shared = dict(wa1t=wa1t, wa2t=wa2t, wa3=wa3, whht=whht, wihb=wihb,
                  bias1=bias1, wffh=wffh, ident=ident, fold64=fold64)

    wfc_x = W_fc[0, :512]
    wff_x = W_ff[0, 512:]
    wfcy = float(W_fc[0, 512])
    bfc = float(np.asarray(inputs["b_fc"], np.float32)[0])

    in_maps = []
    for cix in range(NCORES):
        xs = x[cix * BL:(cix + 1) * BL]                  # (128, 64, 512)
        m = dict(shared)
        xw_c = xs @ wfc_x                                # (128, T)
        xw2_c = xs @ wff_x
        xwp = np.empty((128, 64), np.float32)
        xw2p = np.empty((128, 64), np.float32)
        for S in (0, 1):
            sub = xw_c[S * SB:(S + 1) * SB].reshape(SB, 32, 2)
            xwp[:, S * 32:(S + 1) * 32] = \
                sub.transpose(2, 0, 1).reshape(128, 32)
            sub2 = xw2_c[S * SB:(S + 1) * SB].reshape(SB, 32, 2)
            xw2p[:, S * 32:(S + 1) * 32] = \
                sub2.transpose(2, 0, 1).reshape(128, 32)
        m["xwp"] = xwp
        m["xw2p"] = xw2p
        for S, key in ((0, "xtA"), (1, "xtB")):
            xh = xs[S * SB:(S + 1) * SB]                 # (64, 64, 512)
            xt = (xh.transpose(2, 1, 0).reshape(EC, 128, T * SB)
                  .transpose(1, 0, 2).reshape(128, EC * T * SB).astype(f16))
            m[key] = np.ascontiguousarray(xt)
        yh_c = yh[cix * BL:(cix + 1) * BL, :, 0]          # (128, T)
        ytw_c = yh_c * wfcy + bfc
        ytwt = np.empty((1, 2 * T * SB), np.float32)
        for S in (0, 1):
            ytwt[0, S * T * SB:(S + 1) * T * SB] = \
                ytw_c[S * SB:(S + 1) * SB].T.reshape(-1)
        m["ytwt"] = ytwt
        in_maps.append(m)

    return in_maps


def kernel(**inputs):
    from concourse.bass_utils import run_bass_kernel_spmd

    in_maps = _prep_inputs(inputs)
    W_fc = np.asarray(inputs["W_fc"], dtype=np.float32)
    b_fc = np.asarray(inputs["b_fc"], dtype=np.float32)
    b_ff = np.asarray(inputs["b_ff"], dtype=np.float32)
    wfcy, bfc, bff = float(W_fc[0, 512]), float(b_fc[0]), float(b_ff[0])

    key = (N_STEPS, bff)
    if key not in _PROG_CACHE:
        _PROG_CACHE[key] = _build_program(N_STEPS, bff)
    nc = _PROG_CACHE[key]

    res = run_bass_kernel_spmd(nc, in_maps, core_ids=list(range(NCORES)))
    out = np.concatenate(
        [np.concatenate([res.results[c]["outA"], res.results[c]["outB"]],
                        axis=0)
         for c in range(NCORES)], axis=0).astype(np.float32)
    return out
